# revision 39
# baseline (speedup 1.0000x reference)
"""Trainium2 Bass kernel for nn_AttentionV4 (patch attention, 8 heads on 8 cores).

Pipeline per core (= per head h), v2:
  - The 1x1 qkv conv + depthwise 3x3 conv are fused into one dense matmul over
    a 6x6-windowed patch basis (kappa = (ph, pw, c) in [6,6,48] = 1728,
    chunked 14 x 128), n = interior patch (64x64 grid = 4096; boundary patches
    of the stride-4 pad-4 unfold are exactly zero, handled analytically).
  - All matmul data in bf16 (tolerance 2e-2 gives plenty of headroom); PSUM
    accumulation stays fp32.
  - The windowed rhs (xp) is pre-gathered on the host into a dense per-piece
    layout so each piece is ONE big DMA (the old per-chunk gather serialized
    the sync queue).
  - Column norms of Q/K and V^T transposes are fused into the front-end piece
    loop (no serial normalize phase).
  - Attention: A = Q^T K in [-1,1]*temp scaled per-row by rqt inside the exp;
    E = exp in bf16; Z = rowsum via ACT accumulators (+260 for the zero
    boundary K columns); out = (V/Z) @ E accumulated 4 row-tiles per PSUM
    group, software-pipelined one group ahead of the exp stream.
  - Tail: no collective. Each core applies its head's slice of the final 48x48
    projection (arranged per pixel-phase) to its own full-image output and the
    host sums the 8 partial projections.
"""
import sys
import types

sys.path.insert(0, "/opt/trn_rl_repo")

import numpy as np
import ml_dtypes

BF = ml_dtypes.bfloat16

# ---------------------------------------------------------------- constants
C = 48          # image channels
CH = 6          # channels per head
NH = 8          # heads == cores
GN = 64         # interior patch grid
N = GN * GN     # 4096 interior patches
M96 = 96        # rows of a head matrix (6ch * 4 * 4)
NKAP = 1728     # 36 windows * 48 channels
ZCORR = 260.0   # 4356 - 4096 zero K-columns, exp(0) each
NPIECE = 8      # front-end N pieces (8 patch rows, 512 patches each)
NCORES = 8
NCH14 = 14      # kappa chunks of <=128

# group list (ph, pw) in kappa order: class-major ((dh,dw) in order), then
# hm-major, wm-minor inside the class
_GROUPS = []
for _dh, _dw in [(0, 0), (0, 1), (1, 0), (1, 1)]:
    for _hm in range(4 if _dh == 0 else 2):
        for _wm in range(4 if _dw == 0 else 2):
            _GROUPS.append((_dh * 4 + _hm, _dw * 4 + _wm))

_PHS = np.array([g[0] for g in _GROUPS for _ in range(C)])
_PWS = np.array([g[1] for g in _GROUPS for _ in range(C)])
_CS = np.tile(np.arange(C), NKAP // C)


def _chunk_plan14():
    """14 chunks of K<=128; class-pure (class sizes are multiples of 128).

    Each entry: (krows, dh, dw, runs) with runs = (off, len, hm, wm, c0).
    """
    plan = []
    for k in range(NCH14):
        k0, k1 = 128 * k, min(128 * (k + 1), NKAP)
        runs = []
        kap = k0
        dh = dw = None
        while kap < k1:
            g, c = divmod(kap, C)
            ph, pw = _GROUPS[g]
            if dh is None:
                dh, dw = ph // 4, pw // 4
            assert (ph // 4, pw // 4) == (dh, dw), "chunk crosses class"
            run_end = min((g + 1) * C, k1)
            runs.append((kap - k0, run_end - kap, ph % 4, pw % 4, c))
            kap = run_end
        plan.append((k1 - k0, dh, dw, runs))
    return plan


CHUNKS14 = _chunk_plan14()

# ---------------------------------------------------------------- host prep


def _build_xb(x):
    """Block layout of the (+1,+3)-padded image: xb[hm, wm, c, hq, wq]."""
    xpad = np.zeros((C, 260, 260), np.float32)
    xpad[:, 1:257, 1:257] = x[0]
    return np.ascontiguousarray(
        xpad.reshape(C, 65, 4, 65, 4).transpose(2, 4, 0, 1, 3))


def _build_xp(x):
    """Per-piece dense windowed rhs: xp[piece, 128, 14, 9, 65] bf16."""
    xb = _build_xb(x)           # [4, 4, C, 65, 65]
    xp = np.zeros((NPIECE, 128, NCH14, 9, 65), np.float32)
    for p in range(NPIECE):
        r0 = 8 * p
        for k, (krows, dh, dw, runs) in enumerate(CHUNKS14):
            for (off, ln, hm, wm, c0) in runs:
                xp[p, off:off + ln, k, :, :] = \
                    xb[hm, wm, c0:c0 + ln, r0:r0 + 9, :]
    return np.ascontiguousarray(xp.astype(BF))


def _build_w4(h, w_qkv, w_dw):
    """Fused (1x1 conv + dw3x3) weights in the kappa basis: [128,14,288]."""
    kh = np.arange(4)
    dy = _PHS[:, None] - kh[None, :]            # [1728, 4]
    dx = _PWS[:, None] - kh[None, :]
    my = (dy >= 0) & (dy < 3)
    mx = (dx >= 0) & (dx < 3)
    dyc = np.clip(dy, 0, 2)
    dxc = np.clip(dx, 0, 2)
    w4 = np.zeros((NKAP, 3, CH, 4, 4), np.float32)
    for sel in range(3):
        for cl in range(CH):
            o = sel * C + CH * h + cl
            wd = w_dw[o, 0]
            taps = (wd[dyc[:, :, None], dxc[:, None, :]]
                    * my[:, :, None] * mx[:, None, :])
            w4[:, sel, cl] = w_qkv[o, _CS][:, None, None] * taps
    w4 = w4.reshape(NKAP, 3, M96)
    w4p = np.zeros((128, NCH14, 3, 128), np.float32)
    for k in range(NCH14):
        k0, k1 = 128 * k, min(128 * (k + 1), NKAP)
        w4p[:k1 - k0, k, :, :M96] = w4[k0:k1]
    return np.ascontiguousarray(w4p.reshape(128, NCH14, 384).astype(BF))


def _build_wproj16(h, w_proj):
    """Per-head phase-blocked projection: [96, 8*96] bf16.

    Column block pp2 holds output rows (o + 48*pl) for phases p = 2*pp2 + pl,
    p = 4*kh + kw.  lhsT[(cl,kh,kw), 96*pp2 + 48*pl + o] = w_proj[o, 6h+cl]
    iff (kh,kw) matches phase p.
    """
    w16 = np.zeros((M96, 8 * 128), np.float32)
    for p in range(16):
        kh, kw = p // 4, p % 4
        pp2, pl = p // 2, p % 2
        rows = np.arange(CH) * 16 + 4 * kh + kw          # (cl, kh, kw)
        cols = 128 * pp2 + 48 * pl + np.arange(C)        # (o)
        w16[np.ix_(rows, cols)] = w_proj[:, CH * h:CH * h + CH].T
    return np.ascontiguousarray(w16.astype(BF))


# ---------------------------------------------------------------- program

_PROG = None


def _build_program():
    import antenv  # noqa: F401
    if "antenv.axon_hooks" not in sys.modules:
        holder = {}
        m = types.ModuleType("antenv.axon_hooks")
        m.set_axon_ntff_profile_hook = lambda hk: holder.__setitem__("h", hk)
        m.get_axon_ntff_profile_hook = lambda: holder.get("h")
        sys.modules["antenv.axon_hooks"] = m
        antenv.axon_hooks = m
        try:
            from trn_agent_boot.trn_boot import _ntff_profile_via_ctypes
            m.set_axon_ntff_profile_hook(
                _ntff_profile_via_ctypes("/opt/axon/libaxon_pjrt.so"))
        except Exception:
            pass

    import concourse.bass as bass
    import concourse.tile as tile
    import concourse.mybir as mybir
    from contextlib import ExitStack

    F32 = mybir.dt.float32
    BF16 = mybir.dt.bfloat16
    AF = mybir.ActivationFunctionType

    nc = bass.Bass("TRN2", num_devices=NCORES)

    xp_h = nc.dram_tensor("xp", [NPIECE, 128, NCH14, 9, 65], BF16,
                          kind="ExternalInput")
    w4_h = nc.dram_tensor("w4", [128, NCH14, 384], BF16, kind="ExternalInput")
    vcol_h = nc.dram_tensor("vcol", [M96, 2], BF16, kind="ExternalInput")
    w16_h = nc.dram_tensor("w16", [M96, 8 * 128], BF16, kind="ExternalInput")
    id96_h = nc.dram_tensor("id96", [M96, M96], BF16, kind="ExternalInput")
    onesb_h = nc.dram_tensor("onesb", [1, 128], BF16, kind="ExternalInput")
    y_h = nc.dram_tensor("y", [8, M96, N], BF16, kind="ExternalOutput")

    with tile.TileContext(nc) as tc, ExitStack() as ctx, \
            nc.allow_low_precision(reason="bf16 compute, fp32 accum"):
        const = ctx.enter_context(tc.tile_pool(name="const", bufs=1))
        w4_sb = const.tile([128, NCH14, 384], BF16)
        nc.sync.dma_start(w4_sb[:], w4_h[:])
        vcol_sb = const.tile([M96, 2], BF16)
        nc.sync.dma_start(vcol_sb[:], vcol_h[:])
        w16_sb = const.tile([M96, 8 * 128], BF16)
        nc.sync.dma_start(w16_sb[:], w16_h[:])
        id96_sb = const.tile([M96, M96], BF16)
        nc.sync.dma_start(id96_sb[:], id96_h[:])
        onesb_sb = const.tile([1, 128], BF16)
        nc.sync.dma_start(onesb_sb[:], onesb_h[:])

        persist = ctx.enter_context(tc.tile_pool(name="persist", bufs=1))
        qn = persist.tile([M96, N], BF16)
        kn = persist.tile([M96, N], BF16)
        vt = persist.tile([128, 32 * M96], BF16)
        zacc = persist.tile([128, 96], F32)
        out_acc = persist.tile([M96, N], F32)
        oa_b = persist.tile([M96, N], BF16)

        # ---------------- front end: Q/K/V + norms + V^T, per piece --------
        fe_cm = ExitStack()
        xp_pool = fe_cm.enter_context(tc.tile_pool(name="fe_xp", bufs=3))
        sq_pool = fe_cm.enter_context(tc.tile_pool(name="fe_sq", bufs=2))
        vn_pool = fe_cm.enter_context(tc.tile_pool(name="fe_vn", bufs=2))
        row_pool = fe_cm.enter_context(tc.tile_pool(name="fe_row", bufs=4))
        bkn_pool = fe_cm.enter_context(tc.tile_pool(name="fe_bkn", bufs=2))
        raw_pool = fe_cm.enter_context(tc.tile_pool(name="fe_raw", bufs=3))
        feps = fe_cm.enter_context(
            tc.tile_pool(name="fe_ps", bufs=1, space="PSUM"))
        xp_tiles = {}

        def fetch_xp(p):
            if p < NPIECE:
                xt = xp_pool.tile([128, NCH14, 9, 65], BF16, name="xp",
                                  tag="xp")
                if p == 0:  # split so the first chunk-matmuls start sooner
                    nc.sync.dma_start(xt[:, 0:4], xp_h[p, :, 0:4])
                    nc.sync.dma_start(xt[:, 4:9], xp_h[p, :, 4:9])
                    nc.sync.dma_start(xt[:, 9:NCH14], xp_h[p, :, 9:NCH14])
                else:
                    nc.sync.dma_start(xt[:], xp_h[p])
                xp_tiles[p] = xt

        fetch_xp(0)
        fetch_xp(1)
        for p in range(NPIECE):
            cols = slice(512 * p, 512 * (p + 1))
            fetch_xp(p + 2)
            xp_t = xp_tiles.pop(p)
            raws = {}
            sqs = {}
            vn_p = None
            # all 42 dense chunk-matmuls first: keep the PE stream stall-free
            for sel in range(3):
                ps = feps.tile([128, 512], F32, name="ps", tag="ps", bufs=3)
                for k, (krows, dh, dw, runs) in enumerate(CHUNKS14):
                    nc.tensor.matmul(
                        ps[:],
                        lhsT=w4_sb[:, k, 128 * sel:128 * (sel + 1)],
                        rhs=xp_t[:, k, dh:dh + 8, dw:dw + 64],
                        start=(k == 0), stop=(k == NCH14 - 1))
                if sel < 2:
                    raw = raw_pool.tile([M96, 512], BF16, name="raw",
                                        tag="raw")
                    nc.vector.tensor_copy(raw[:], ps[0:M96, :])
                    raws[sel] = raw
                    sq = sq_pool.tile([M96, 512], BF16, name="sq", tag="sq")
                    nc.vector.tensor_mul(sq[:], raw[:], raw[:])
                    sqs[sel] = sq
                else:
                    vn_p = vn_pool.tile([M96, 512], BF16, name="vn", tag="vn")
                    nc.vector.tensor_copy(vn_p[:], ps[0:M96, :])
            # small dependent matmuls + transposes at piece end
            rsqs = {}
            for sel in range(2):
                ssp = feps.tile([1, 512], F32, name="ssp", tag="ssp", bufs=2)
                nc.tensor.matmul(ssp[:], lhsT=vcol_sb[:, sel:sel + 1],
                                 rhs=sqs[sel][:], start=True, stop=True)
                lnr = row_pool.tile([1, 512], F32, name="lnr", tag="lnr")
                nc.scalar.activation(lnr[:], ssp[:], AF.Ln)
                rsq = row_pool.tile([1, 512], BF16, name="rsq", tag="rsq")
                nc.scalar.activation(rsq[:], lnr[:], AF.Exp, scale=-0.5)
                rsqs[sel] = rsq
            for half in range(4):
                ti = 4 * p + half
                tp = feps.tile([128, M96], BF16, name="tp", tag="tp", bufs=1)
                nc.tensor.transpose(
                    tp[:], vn_p[:, 128 * half:128 * (half + 1)], id96_sb[:])
                nc.vector.tensor_copy(vt[:, M96 * ti:M96 * (ti + 1)], tp[:])
            bkps = feps.tile([128, 1024], F32, name="bkps", tag="bkps",
                             bufs=1)
            for sel in range(2):
                nc.tensor.matmul(bkps[:, 512 * sel:512 * (sel + 1)],
                                 lhsT=onesb_sb[:], rhs=rsqs[sel][:],
                                 start=True, stop=True)
            bkn = bkn_pool.tile([M96, 1024], BF16, name="bkn", tag="bkn")
            nc.scalar.activation(bkn[:], bkps[0:M96, :], AF.Copy)
            nc.vector.tensor_mul(qn[:, cols], raws[0][:], bkn[:, 0:512])
            nc.vector.tensor_mul(kn[:, cols], raws[1][:], bkn[:, 512:1024])
        fe_cm.close()

        # ---------------- attention, pipelined one group ahead ------------
        at_cm = ExitStack()
        ps_cm = ExitStack()
        papool = ps_cm.enter_context(
            tc.tile_pool(name="a_pa", bufs=1, space="PSUM"))
        pbpool = ps_cm.enter_context(
            tc.tile_pool(name="a_pb", bufs=1, space="PSUM"))
        opool = ps_cm.enter_context(
            tc.tile_pool(name="a_op", bufs=2, space="PSUM"))
        espool = at_cm.enter_context(tc.tile_pool(name="a_es", bufs=8))
        vtspool = at_cm.enter_context(tc.tile_pool(name="a_vts", bufs=8))
        zpool = at_cm.enter_context(tc.tile_pool(name="a_z", bufs=2))

        es_tiles = {}
        vts_tiles = {}

        def emit_out(g2, j0, nj, pool):
            for j in range(j0, j0 + nj):
                cj = slice(512 * j, 512 * (j + 1))
                op = pool.tile([M96, 512], F32, name="op", tag="op")
                for tl2 in range(4):
                    t2 = 4 * g2 + tl2
                    nc.tensor.matmul(
                        op[:], lhsT=vts_tiles[t2],
                        rhs=es_tiles[t2][:, cj],
                        start=(tl2 == 0), stop=(tl2 == 3))
                if g2 == 0:
                    nc.vector.tensor_copy(out_acc[:, cj], op[:])
                else:
                    nc.vector.tensor_add(out_acc[:, cj], out_acc[:, cj],
                                         op[:])
                if g2 == 7:
                    nc.vector.tensor_copy(oa_b[:, cj], out_acc[:, cj])

        MSPLITS = ((0, 1536), (1536, 1536), (3072, 1024))
        arenaA = papool.tile([128, 1536], F32, name="arA")
        arenaB = pbpool.tile([128, 1536], F32, name="arB")
        for g in range(8):
            for tl in range(4):
                t = 4 * g + tl
                es = espool.tile([128, N], BF16, name="es", tag="es")
                es_tiles[t] = es
                regs = (arenaA, arenaB, arenaA) if t % 2 == 0 else \
                    (arenaB, arenaA, arenaB)
                for si, (m0, mw) in enumerate(MSPLITS):
                    pa = regs[si]
                    for i in range(mw // 512):
                        nc.tensor.matmul(
                            pa[:, 512 * i:512 * (i + 1)],
                            lhsT=qn[:, 128 * t:128 * (t + 1)],
                            rhs=kn[:, m0 + 512 * i:m0 + 512 * (i + 1)],
                            start=True, stop=True)
                    if si < 2:
                        nc.scalar.activation(
                            es[:, m0:m0 + mw], pa[:, 0:mw], AF.Exp,
                            accum_out=zacc[:, 3 * t + si:3 * t + si + 1])
                    else:
                        # last split: Z contribution on the idle DVE instead
                        nc.scalar.activation(
                            es[:, m0:m0 + mw], pa[:, 0:mw], AF.Exp)
                        nc.vector.tensor_reduce(
                            zacc[:, 3 * t + 2:3 * t + 3], es[:, m0:m0 + mw],
                            axis=mybir.AxisListType.X, op=mybir.AluOpType.add)
                    # (3,3,2,0) j-distribution: frees the previous group's
                    # es tiles one slot earlier so g+1's first A-matmuls
                    # don't wait on the es ring
                    if si == 1 and g > 0 and tl < 3:
                        emit_out(g - 1, (0, 3, 6)[tl], 1, opool)
                if g > 0 and tl < 3:
                    nj = (2, 2, 1)[tl]
                    emit_out(g - 1, (1, 4, 7)[tl], nj, opool)
                if g == 7:
                    # per-t epilogue so the tail can start right after exp(31)
                    zs = zpool.tile([128, 1], F32, name="zs1", tag="zs1")
                    nc.vector.tensor_reduce(
                        zs[:],
                        zacc[:, 3 * t:3 * t + 3].rearrange(
                            "p (o x) -> p o x", o=1),
                        axis=mybir.AxisListType.X, op=mybir.AluOpType.add)
                    nc.vector.tensor_scalar_add(zs[:], zs[:], ZCORR)
                    nc.vector.reciprocal(zs[:], zs[:])
                    vts = vtspool.tile([128, M96], BF16, name="vts",
                                       tag="vts")
                    nc.vector.tensor_scalar_mul(
                        vts[:], vt[:, M96 * t:M96 * (t + 1)], zs[:])
                    vts_tiles[t] = vts
            if g < 7:
                # group epilogue: zinv + vts
                zs = zpool.tile([128, 4], F32, name="zs", tag="zs")
                nc.vector.tensor_reduce(
                    zs[:],
                    zacc[:, 12 * g:12 * (g + 1)].rearrange(
                        "p (t x) -> p t x", t=4),
                    axis=mybir.AxisListType.X, op=mybir.AluOpType.add)
                nc.vector.tensor_scalar_add(zs[:], zs[:], ZCORR)
                nc.vector.reciprocal(zs[:], zs[:])
                for tl in range(4):
                    t = 4 * g + tl
                    vts = vtspool.tile([128, M96], BF16, name="vts",
                                       tag="vts")
                    nc.vector.tensor_scalar_mul(
                        vts[:], vt[:, M96 * t:M96 * (t + 1)],
                        zs[:, tl:tl + 1])
                    vts_tiles[t] = vts
        ps_cm.close()

        # ------- last group's out-mms interleaved with projection tail ----
        with tc.tile_pool(name="t_op", bufs=2, space="PSUM") as op2, \
                tc.tile_pool(name="prj_ps", bufs=4, space="PSUM") as prjps, \
                tc.tile_pool(name="yt", bufs=2) as ypool:
            for jn in range(8):
                cj = slice(512 * jn, 512 * (jn + 1))
                emit_out(7, jn, 1, op2)
                yt = ypool.tile([M96, N], BF16, name="yt", tag="yt")
                for pp2 in range(8):
                    cb = slice(512 * pp2, 512 * (pp2 + 1))
                    pp_ps = prjps.tile([128, 512], F32, name="pps", tag="pps")
                    nc.tensor.matmul(
                        pp_ps[:], lhsT=w16_sb[:, 128 * pp2:128 * (pp2 + 1)],
                        rhs=oa_b[:, cj], start=True, stop=True)
                    if pp2 % 2 == 0:
                        nc.vector.tensor_copy(yt[:, cb], pp_ps[0:M96, :])
                    else:
                        nc.scalar.activation(yt[:, cb], pp_ps[0:M96, :],
                                             AF.Copy)
                nc.sync.dma_start(y_h[jn], yt[:])
        at_cm.close()

    _split_excess_waits(nc)
    return nc


_wsplit_ctr = [0]


def _split_excess_waits(nc, max_waits=1):
    """This walrus build encodes only one sync-wait per instruction; hoist
    extras onto same-engine nops inserted directly before the instruction."""
    import bass_rust
    import concourse.mybir as mybir
    for fn in nc.m.functions:
        for bb in fn.blocks:
            insts = bb.instructions
            out = []
            changed = False
            for inst in insts:
                si = inst.sync_info
                if si is not None and len(si.on_wait) > max_waits:
                    waits = list(si.on_wait)
                    for w in waits[:-max_waits]:
                        _wsplit_ctr[0] += 1
                        nop = bass_rust.InstNoOp(
                            name=f"I-wsplit-{_wsplit_ctr[0]}", ins=[], outs=[])
                        nop.engine = inst.engine
                        nop.sync_info = mybir.SyncInfo(
                            on_wait=[w], on_update=[])
                        out.append(nop)
                    inst.sync_info = mybir.SyncInfo(
                        on_wait=waits[-max_waits:],
                        on_update=list(si.on_update))
                    changed = True
                out.append(inst)
            if changed:
                bb.instructions = out


def _get_program():
    global _PROG
    if _PROG is None:
        _PROG = _build_program()
    return _PROG


# ---------------------------------------------------------------- entry

def kernel(x, w_qkv, w_dw, temperature, w_proj, _trace=False):
    x = np.asarray(x, np.float32)
    w_qkv = np.asarray(w_qkv, np.float32)
    w_dw = np.asarray(w_dw, np.float32)
    temperature = np.asarray(temperature, np.float32)
    w_proj = np.asarray(w_proj, np.float32)

    nc = _get_program()
    from concourse.bass_utils import run_bass_kernel_spmd

    xp = _build_xp(x)
    id96 = np.eye(M96, dtype=BF)
    onesb = np.ones((1, 128), BF)
    in_maps = []
    for h in range(NH):
        t_h = float(temperature[h, 0, 0])
        vcol = np.empty((M96, 2), np.float32)
        vcol[:, 0] = 1.0 / (t_h * t_h)
        vcol[:, 1] = 1.0
        in_maps.append({
            "xp": xp,
            "w4": _build_w4(h, w_qkv, w_dw),
            "vcol": vcol.astype(BF),
            "w16": _build_wproj16(h, w_proj),
            "id96": id96,
            "onesb": onesb,
        })

    res = run_bass_kernel_spmd(nc, in_maps, list(range(NCORES)), trace=_trace)

    # host gather: sum per-head partial projections, then phase reassembly
    acc = np.zeros((8, M96, N), np.float32)
    for s in range(NCORES):
        acc += np.asarray(res.results[s]["y"]).astype(np.float32)
    # acc[jn, (pl, o), (pp2, i)] -> [p = (pp2, pl), o, n = (jn, i)]
    acc = acc.reshape(8, 2, C, 8, 512).transpose(3, 1, 2, 0, 4)
    acc = acc.reshape(16, C, GN, GN)
    y = np.ascontiguousarray(
        acc.reshape(4, 4, C, GN, GN).transpose(2, 3, 0, 4, 1)
        .reshape(C, 256, 256))[None]
    if _trace:
        return y, res
    return y


# revision 44
# speedup vs baseline: 1.0148x; 1.0148x over previous
"""Trainium2 Bass kernel for nn_AttentionV4 (patch attention, 8 heads on 8 cores).

Pipeline per core (= per head h), v2:
  - The 1x1 qkv conv + depthwise 3x3 conv are fused into one dense matmul over
    a 6x6-windowed patch basis (kappa = (ph, pw, c) in [6,6,48] = 1728,
    chunked 14 x 128), n = interior patch (64x64 grid = 4096; boundary patches
    of the stride-4 pad-4 unfold are exactly zero, handled analytically).
  - All matmul data in bf16 (tolerance 2e-2 gives plenty of headroom); PSUM
    accumulation stays fp32.
  - The windowed rhs (xp) is pre-gathered on the host into a dense per-piece
    layout so each piece is ONE big DMA (the old per-chunk gather serialized
    the sync queue).
  - Column norms of Q/K and V^T transposes are fused into the front-end piece
    loop (no serial normalize phase).
  - Attention: A = Q^T K in [-1,1]*temp scaled per-row by rqt inside the exp;
    E = exp in bf16; Z = rowsum via ACT accumulators (+260 for the zero
    boundary K columns); out = (V/Z) @ E accumulated 4 row-tiles per PSUM
    group, software-pipelined one group ahead of the exp stream.
  - Tail: no collective. Each core applies its head's slice of the final 48x48
    projection (arranged per pixel-phase) to its own full-image output and the
    host sums the 8 partial projections.
"""
import sys
import types

sys.path.insert(0, "/opt/trn_rl_repo")

import numpy as np
import ml_dtypes

BF = ml_dtypes.bfloat16

# ---------------------------------------------------------------- constants
C = 48          # image channels
CH = 6          # channels per head
NH = 8          # heads == cores
GN = 64         # interior patch grid
N = GN * GN     # 4096 interior patches
M96 = 96        # rows of a head matrix (6ch * 4 * 4)
NKAP = 1728     # 36 windows * 48 channels
ZCORR = 260.0   # 4356 - 4096 zero K-columns, exp(0) each
NPIECE = 8      # front-end N pieces (8 patch rows, 512 patches each)
NCORES = 8
NCH14 = 14      # kappa chunks of <=128

# group list (ph, pw) in kappa order: class-major ((dh,dw) in order), then
# hm-major, wm-minor inside the class
_GROUPS = []
for _dh, _dw in [(0, 0), (0, 1), (1, 0), (1, 1)]:
    for _hm in range(4 if _dh == 0 else 2):
        for _wm in range(4 if _dw == 0 else 2):
            _GROUPS.append((_dh * 4 + _hm, _dw * 4 + _wm))

_PHS = np.array([g[0] for g in _GROUPS for _ in range(C)])
_PWS = np.array([g[1] for g in _GROUPS for _ in range(C)])
_CS = np.tile(np.arange(C), NKAP // C)


def _chunk_plan14():
    """14 chunks of K<=128; class-pure (class sizes are multiples of 128).

    Each entry: (krows, dh, dw, runs) with runs = (off, len, hm, wm, c0).
    """
    plan = []
    for k in range(NCH14):
        k0, k1 = 128 * k, min(128 * (k + 1), NKAP)
        runs = []
        kap = k0
        dh = dw = None
        while kap < k1:
            g, c = divmod(kap, C)
            ph, pw = _GROUPS[g]
            if dh is None:
                dh, dw = ph // 4, pw // 4
            assert (ph // 4, pw // 4) == (dh, dw), "chunk crosses class"
            run_end = min((g + 1) * C, k1)
            runs.append((kap - k0, run_end - kap, ph % 4, pw % 4, c))
            kap = run_end
        plan.append((k1 - k0, dh, dw, runs))
    return plan


CHUNKS14 = _chunk_plan14()

# ---------------------------------------------------------------- host prep


def _build_xb(x):
    """Block layout of the (+1,+3)-padded image: xb[hm, wm, c, hq, wq]."""
    xpad = np.zeros((C, 260, 260), np.float32)
    xpad[:, 1:257, 1:257] = x[0]
    return np.ascontiguousarray(
        xpad.reshape(C, 65, 4, 65, 4).transpose(2, 4, 0, 1, 3))


def _build_xp(x):
    """Per-piece dense windowed rhs: xp[piece, 128, 14, 9, 65] bf16."""
    xb = _build_xb(x)           # [4, 4, C, 65, 65]
    xp = np.zeros((NPIECE, 128, NCH14, 9, 65), np.float32)
    for p in range(NPIECE):
        r0 = 8 * p
        for k, (krows, dh, dw, runs) in enumerate(CHUNKS14):
            for (off, ln, hm, wm, c0) in runs:
                xp[p, off:off + ln, k, :, :] = \
                    xb[hm, wm, c0:c0 + ln, r0:r0 + 9, :]
    return np.ascontiguousarray(xp.astype(BF))


def _build_w4(h, w_qkv, w_dw):
    """Fused (1x1 conv + dw3x3) weights in the kappa basis: [128,14,288]."""
    kh = np.arange(4)
    dy = _PHS[:, None] - kh[None, :]            # [1728, 4]
    dx = _PWS[:, None] - kh[None, :]
    my = (dy >= 0) & (dy < 3)
    mx = (dx >= 0) & (dx < 3)
    dyc = np.clip(dy, 0, 2)
    dxc = np.clip(dx, 0, 2)
    w4 = np.zeros((NKAP, 3, CH, 4, 4), np.float32)
    for sel in range(3):
        for cl in range(CH):
            o = sel * C + CH * h + cl
            wd = w_dw[o, 0]
            taps = (wd[dyc[:, :, None], dxc[:, None, :]]
                    * my[:, :, None] * mx[:, None, :])
            w4[:, sel, cl] = w_qkv[o, _CS][:, None, None] * taps
    w4 = w4.reshape(NKAP, 3, M96)
    w4p = np.zeros((128, NCH14, 3, 128), np.float32)
    for k in range(NCH14):
        k0, k1 = 128 * k, min(128 * (k + 1), NKAP)
        w4p[:k1 - k0, k, :, :M96] = w4[k0:k1]
    return np.ascontiguousarray(w4p.reshape(128, NCH14, 384).astype(BF))


def _build_wproj16(h, w_proj):
    """Per-head phase-blocked projection: [96, 8*96] bf16.

    Column block pp2 holds output rows (o + 48*pl) for phases p = 2*pp2 + pl,
    p = 4*kh + kw.  lhsT[(cl,kh,kw), 96*pp2 + 48*pl + o] = w_proj[o, 6h+cl]
    iff (kh,kw) matches phase p.
    """
    w16 = np.zeros((M96, 8 * 128), np.float32)
    for p in range(16):
        kh, kw = p // 4, p % 4
        pp2, pl = p // 2, p % 2
        rows = np.arange(CH) * 16 + 4 * kh + kw          # (cl, kh, kw)
        cols = 128 * pp2 + 48 * pl + np.arange(C)        # (o)
        w16[np.ix_(rows, cols)] = w_proj[:, CH * h:CH * h + CH].T
    return np.ascontiguousarray(w16.astype(BF))


# ---------------------------------------------------------------- program

_PROG = None


def _build_program():
    import antenv  # noqa: F401
    if "antenv.axon_hooks" not in sys.modules:
        holder = {}
        m = types.ModuleType("antenv.axon_hooks")
        m.set_axon_ntff_profile_hook = lambda hk: holder.__setitem__("h", hk)
        m.get_axon_ntff_profile_hook = lambda: holder.get("h")
        sys.modules["antenv.axon_hooks"] = m
        antenv.axon_hooks = m
        try:
            from trn_agent_boot.trn_boot import _ntff_profile_via_ctypes
            m.set_axon_ntff_profile_hook(
                _ntff_profile_via_ctypes("/opt/axon/libaxon_pjrt.so"))
        except Exception:
            pass

    import concourse.bass as bass
    import concourse.tile as tile
    import concourse.mybir as mybir
    from contextlib import ExitStack

    F32 = mybir.dt.float32
    BF16 = mybir.dt.bfloat16
    AF = mybir.ActivationFunctionType

    nc = bass.Bass("TRN2", num_devices=NCORES)

    xp_h = nc.dram_tensor("xp", [NPIECE, 128, NCH14, 9, 65], BF16,
                          kind="ExternalInput")
    w4_h = nc.dram_tensor("w4", [128, NCH14, 384], BF16, kind="ExternalInput")
    vcol_h = nc.dram_tensor("vcol", [M96, 2], BF16, kind="ExternalInput")
    w16_h = nc.dram_tensor("w16", [M96, 8 * 128], BF16, kind="ExternalInput")
    id96_h = nc.dram_tensor("id96", [M96, M96], BF16, kind="ExternalInput")
    onesb_h = nc.dram_tensor("onesb", [1, 128], BF16, kind="ExternalInput")
    y_h = nc.dram_tensor("y", [8, M96, N], BF16, kind="ExternalOutput")

    with tile.TileContext(nc) as tc, ExitStack() as ctx, \
            nc.allow_low_precision(reason="bf16 compute, fp32 accum"):
        const = ctx.enter_context(tc.tile_pool(name="const", bufs=1))
        w4_sb = const.tile([128, NCH14, 384], BF16)
        nc.sync.dma_start(w4_sb[:], w4_h[:])
        vcol_sb = const.tile([M96, 2], BF16)
        nc.sync.dma_start(vcol_sb[:], vcol_h[:])
        w16_sb = const.tile([M96, 8 * 128], BF16)
        nc.sync.dma_start(w16_sb[:], w16_h[:])
        id96_sb = const.tile([M96, M96], BF16)
        nc.sync.dma_start(id96_sb[:], id96_h[:])
        onesb_sb = const.tile([1, 128], BF16)
        nc.sync.dma_start(onesb_sb[:], onesb_h[:])

        persist = ctx.enter_context(tc.tile_pool(name="persist", bufs=1))
        qn = persist.tile([M96, N], BF16)
        kn = persist.tile([M96, N], BF16)
        vt = persist.tile([128, 32 * M96], BF16)
        zacc = persist.tile([128, 96], F32)
        out_acc = persist.tile([M96, N], F32)
        oa_b = persist.tile([M96, N], BF16)

        # ---------------- front end: Q/K/V + norms + V^T, per piece --------
        fe_cm = ExitStack()
        xp_pool = fe_cm.enter_context(tc.tile_pool(name="fe_xp", bufs=3))
        sq_pool = fe_cm.enter_context(tc.tile_pool(name="fe_sq", bufs=2))
        vn_pool = fe_cm.enter_context(tc.tile_pool(name="fe_vn", bufs=2))
        row_pool = fe_cm.enter_context(tc.tile_pool(name="fe_row", bufs=4))
        bkn_pool = fe_cm.enter_context(tc.tile_pool(name="fe_bkn", bufs=2))
        raw_pool = fe_cm.enter_context(tc.tile_pool(name="fe_raw", bufs=3))
        feps = fe_cm.enter_context(
            tc.tile_pool(name="fe_ps", bufs=1, space="PSUM"))
        xp_tiles = {}

        def fetch_xp(p):
            if p < NPIECE:
                xt = xp_pool.tile([128, NCH14, 9, 65], BF16, name="xp",
                                  tag="xp")
                if p == 0:  # split so the first chunk-matmuls start sooner
                    nc.sync.dma_start(xt[:, 0:4], xp_h[p, :, 0:4])
                    nc.sync.dma_start(xt[:, 4:9], xp_h[p, :, 4:9])
                    nc.sync.dma_start(xt[:, 9:NCH14], xp_h[p, :, 9:NCH14])
                else:
                    nc.sync.dma_start(xt[:], xp_h[p])
                xp_tiles[p] = xt

        fetch_xp(0)
        fetch_xp(1)
        for p in range(NPIECE):
            cols = slice(512 * p, 512 * (p + 1))
            fetch_xp(p + 2)
            xp_t = xp_tiles.pop(p)
            raws = {}
            sqs = {}
            vn_p = None
            # all 42 dense chunk-matmuls first: keep the PE stream stall-free
            for sel in range(3):
                ps = feps.tile([128, 512], F32, name="ps", tag="ps", bufs=3)
                for k, (krows, dh, dw, runs) in enumerate(CHUNKS14):
                    nc.tensor.matmul(
                        ps[:],
                        lhsT=w4_sb[:, k, 128 * sel:128 * (sel + 1)],
                        rhs=xp_t[:, k, dh:dh + 8, dw:dw + 64],
                        start=(k == 0), stop=(k == NCH14 - 1))
                if sel < 2:
                    raw = raw_pool.tile([M96, 512], BF16, name="raw",
                                        tag="raw")
                    nc.vector.tensor_copy(raw[:], ps[0:M96, :])
                    raws[sel] = raw
                    sq = sq_pool.tile([M96, 512], BF16, name="sq", tag="sq")
                    nc.vector.tensor_mul(sq[:], raw[:], raw[:])
                    sqs[sel] = sq
                else:
                    vn_p = vn_pool.tile([M96, 512], BF16, name="vn", tag="vn")
                    nc.vector.tensor_copy(vn_p[:], ps[0:M96, :])
            # small dependent matmuls + transposes at piece end
            rsqs = {}
            for sel in range(2):
                ssp = feps.tile([1, 512], F32, name="ssp", tag="ssp", bufs=2)
                nc.tensor.matmul(ssp[:], lhsT=vcol_sb[:, sel:sel + 1],
                                 rhs=sqs[sel][:], start=True, stop=True)
                lnr = row_pool.tile([1, 512], F32, name="lnr", tag="lnr")
                nc.scalar.activation(lnr[:], ssp[:], AF.Ln)
                rsq = row_pool.tile([1, 512], BF16, name="rsq", tag="rsq")
                nc.scalar.activation(rsq[:], lnr[:], AF.Exp, scale=-0.5)
                rsqs[sel] = rsq
            for half in range(4):
                ti = 4 * p + half
                tp = feps.tile([128, M96], BF16, name="tp", tag="tp", bufs=1)
                nc.tensor.transpose(
                    tp[:], vn_p[:, 128 * half:128 * (half + 1)], id96_sb[:])
                nc.vector.tensor_copy(vt[:, M96 * ti:M96 * (ti + 1)], tp[:])
            bkps = feps.tile([128, 1024], F32, name="bkps", tag="bkps",
                             bufs=1)
            for sel in range(2):
                nc.tensor.matmul(bkps[:, 512 * sel:512 * (sel + 1)],
                                 lhsT=onesb_sb[:], rhs=rsqs[sel][:],
                                 start=True, stop=True)
            bkn = bkn_pool.tile([M96, 1024], BF16, name="bkn", tag="bkn")
            nc.scalar.activation(bkn[:], bkps[0:M96, :], AF.Copy)
            nc.vector.tensor_mul(qn[:, cols], raws[0][:], bkn[:, 0:512])
            nc.vector.tensor_mul(kn[:, cols], raws[1][:], bkn[:, 512:1024])
        fe_cm.close()

        # ---------------- attention, pipelined one group ahead ------------
        at_cm = ExitStack()
        ps_cm = ExitStack()
        papool = ps_cm.enter_context(
            tc.tile_pool(name="a_pa", bufs=1, space="PSUM"))
        pbpool = ps_cm.enter_context(
            tc.tile_pool(name="a_pb", bufs=1, space="PSUM"))
        opool = ps_cm.enter_context(
            tc.tile_pool(name="a_op", bufs=2, space="PSUM"))
        espool = at_cm.enter_context(tc.tile_pool(name="a_es", bufs=8))
        vtspool = at_cm.enter_context(tc.tile_pool(name="a_vts", bufs=8))
        zpool = at_cm.enter_context(tc.tile_pool(name="a_z", bufs=2))

        es_tiles = {}
        vts_tiles = {}

        def emit_out(g2, j0, nj, pool):
            for j in range(j0, j0 + nj):
                cj = slice(512 * j, 512 * (j + 1))
                op = pool.tile([M96, 512], F32, name="op", tag="op")
                for tl2 in range(4):
                    t2 = 4 * g2 + tl2
                    nc.tensor.matmul(
                        op[:], lhsT=vts_tiles[t2],
                        rhs=es_tiles[t2][:, cj],
                        start=(tl2 == 0), stop=(tl2 == 3))
                if g2 == 0:
                    nc.vector.tensor_copy(out_acc[:, cj], op[:])
                else:
                    nc.vector.tensor_add(out_acc[:, cj], out_acc[:, cj],
                                         op[:])
                if g2 == 7:
                    nc.vector.tensor_copy(oa_b[:, cj], out_acc[:, cj])

        MSPLITS = ((0, 1536), (1536, 1536), (3072, 1024))
        arenaA = papool.tile([128, 1536], F32, name="arA")
        arenaB = pbpool.tile([128, 1536], F32, name="arB")
        for g in range(8):
            for tl in range(4):
                t = 4 * g + tl
                es = espool.tile([128, N], BF16, name="es", tag="es")
                es_tiles[t] = es
                regs = (arenaA, arenaB, arenaA) if t % 2 == 0 else \
                    (arenaB, arenaA, arenaB)
                for si, (m0, mw) in enumerate(MSPLITS):
                    pa = regs[si]
                    for i in range(mw // 512):
                        nc.tensor.matmul(
                            pa[:, 512 * i:512 * (i + 1)],
                            lhsT=qn[:, 128 * t:128 * (t + 1)],
                            rhs=kn[:, m0 + 512 * i:m0 + 512 * (i + 1)],
                            start=True, stop=True)
                    if si < 2:
                        nc.scalar.activation(
                            es[:, m0:m0 + mw], pa[:, 0:mw], AF.Exp,
                            accum_out=zacc[:, 3 * t + si:3 * t + si + 1])
                    else:
                        # last split: Z contribution on the idle DVE instead
                        nc.scalar.activation(
                            es[:, m0:m0 + mw], pa[:, 0:mw], AF.Exp)
                        nc.vector.tensor_reduce(
                            zacc[:, 3 * t + 2:3 * t + 3], es[:, m0:m0 + mw],
                            axis=mybir.AxisListType.X, op=mybir.AluOpType.add)
                    if si == 1 and g > 0:
                        emit_out(g - 1, 2 * tl, 1, opool)
                if g > 0:
                    emit_out(g - 1, 2 * tl + 1, 1, opool)
                if g == 7:
                    # per-t epilogue so the tail can start right after exp(31)
                    zs = zpool.tile([128, 1], F32, name="zs1", tag="zs1")
                    nc.vector.tensor_reduce(
                        zs[:],
                        zacc[:, 3 * t:3 * t + 3].rearrange(
                            "p (o x) -> p o x", o=1),
                        axis=mybir.AxisListType.X, op=mybir.AluOpType.add)
                    nc.vector.tensor_scalar_add(zs[:], zs[:], ZCORR)
                    nc.vector.reciprocal(zs[:], zs[:])
                    vts = vtspool.tile([128, M96], BF16, name="vts",
                                       tag="vts")
                    nc.vector.tensor_scalar_mul(
                        vts[:], vt[:, M96 * t:M96 * (t + 1)], zs[:])
                    vts_tiles[t] = vts
            if g < 7:
                # group epilogue: zinv + vts
                zs = zpool.tile([128, 4], F32, name="zs", tag="zs")
                nc.vector.tensor_reduce(
                    zs[:],
                    zacc[:, 12 * g:12 * (g + 1)].rearrange(
                        "p (t x) -> p t x", t=4),
                    axis=mybir.AxisListType.X, op=mybir.AluOpType.add)
                nc.vector.tensor_scalar_add(zs[:], zs[:], ZCORR)
                nc.vector.reciprocal(zs[:], zs[:])
                for tl in range(4):
                    t = 4 * g + tl
                    vts = vtspool.tile([128, M96], BF16, name="vts",
                                       tag="vts")
                    nc.vector.tensor_scalar_mul(
                        vts[:], vt[:, M96 * t:M96 * (t + 1)],
                        zs[:, tl:tl + 1])
                    vts_tiles[t] = vts
        ps_cm.close()

        # ------- last group's out-mms interleaved with projection tail ----
        with tc.tile_pool(name="t_op", bufs=2, space="PSUM") as op2, \
                tc.tile_pool(name="prj_ps", bufs=4, space="PSUM") as prjps, \
                tc.tile_pool(name="yt", bufs=2) as ypool:
            for jn in range(8):
                cj = slice(512 * jn, 512 * (jn + 1))
                emit_out(7, jn, 1, op2)
                yt = ypool.tile([M96, N], BF16, name="yt", tag="yt")
                for pp2 in range(8):
                    cb = slice(512 * pp2, 512 * (pp2 + 1))
                    pp_ps = prjps.tile([128, 512], F32, name="pps", tag="pps")
                    nc.tensor.matmul(
                        pp_ps[:], lhsT=w16_sb[:, 128 * pp2:128 * (pp2 + 1)],
                        rhs=oa_b[:, cj], start=True, stop=True)
                    if pp2 % 2 == 0:
                        nc.vector.tensor_copy(yt[:, cb], pp_ps[0:M96, :])
                    else:
                        nc.scalar.activation(yt[:, cb], pp_ps[0:M96, :],
                                             AF.Copy)
                nc.sync.dma_start(y_h[jn], yt[:])
        at_cm.close()

    _split_excess_waits(nc)
    return nc


_wsplit_ctr = [0]


def _split_excess_waits(nc, max_waits=1):
    """This walrus build encodes only one sync-wait per instruction; hoist
    extras onto same-engine nops inserted directly before the instruction."""
    import bass_rust
    import concourse.mybir as mybir
    for fn in nc.m.functions:
        for bb in fn.blocks:
            insts = bb.instructions
            out = []
            changed = False
            for inst in insts:
                si = inst.sync_info
                if si is not None and len(si.on_wait) > max_waits:
                    waits = list(si.on_wait)
                    for w in waits[:-max_waits]:
                        _wsplit_ctr[0] += 1
                        nop = bass_rust.InstNoOp(
                            name=f"I-wsplit-{_wsplit_ctr[0]}", ins=[], outs=[])
                        nop.engine = inst.engine
                        nop.sync_info = mybir.SyncInfo(
                            on_wait=[w], on_update=[])
                        out.append(nop)
                    inst.sync_info = mybir.SyncInfo(
                        on_wait=waits[-max_waits:],
                        on_update=list(si.on_update))
                    changed = True
                out.append(inst)
            if changed:
                bb.instructions = out


def _get_program():
    global _PROG
    if _PROG is None:
        _PROG = _build_program()
    return _PROG


# ---------------------------------------------------------------- entry

def kernel(x, w_qkv, w_dw, temperature, w_proj, _trace=False):
    x = np.asarray(x, np.float32)
    w_qkv = np.asarray(w_qkv, np.float32)
    w_dw = np.asarray(w_dw, np.float32)
    temperature = np.asarray(temperature, np.float32)
    w_proj = np.asarray(w_proj, np.float32)

    nc = _get_program()
    from concourse.bass_utils import run_bass_kernel_spmd

    xp = _build_xp(x)
    id96 = np.eye(M96, dtype=BF)
    onesb = np.ones((1, 128), BF)
    in_maps = []
    for h in range(NH):
        t_h = float(temperature[h, 0, 0])
        vcol = np.empty((M96, 2), np.float32)
        vcol[:, 0] = 1.0 / (t_h * t_h)
        vcol[:, 1] = 1.0
        in_maps.append({
            "xp": xp,
            "w4": _build_w4(h, w_qkv, w_dw),
            "vcol": vcol.astype(BF),
            "w16": _build_wproj16(h, w_proj),
            "id96": id96,
            "onesb": onesb,
        })

    res = run_bass_kernel_spmd(nc, in_maps, list(range(NCORES)), trace=_trace)

    # host gather: sum per-head partial projections, then phase reassembly
    acc = np.zeros((8, M96, N), np.float32)
    for s in range(NCORES):
        acc += np.asarray(res.results[s]["y"]).astype(np.float32)
    # acc[jn, (pl, o), (pp2, i)] -> [p = (pp2, pl), o, n = (jn, i)]
    acc = acc.reshape(8, 2, C, 8, 512).transpose(3, 1, 2, 0, 4)
    acc = acc.reshape(16, C, GN, GN)
    y = np.ascontiguousarray(
        acc.reshape(4, 4, C, GN, GN).transpose(2, 3, 0, 4, 1)
        .reshape(C, 256, 256))[None]
    if _trace:
        return y, res
    return y
